# revision 1
# baseline (speedup 1.0000x reference)
"""Viterbi decode kernel for TRN2 (Bass/Tile) — custom-DVE fused version.

Layout (per core, B_loc=16 batch rows):
  partition p = b*8 + ch   (b in [0,16), ch in [0,8));  cur = ch*16 + cl
  TRW  [128, 16, 128] f32 : TRW[b*8+ch, cl, q] = trans[q, ch*16+cl]
  POT  [128, T*16]   f32 : POT[b*8+ch, t*16+cl]  = pot[b, t, ch*16+cl]
  AHIST DRAM [128, T*16] : alpha_t[b, cur] = AHIST[b*8+ch, t*16+cl]

Forward per t: ONE fused custom-DVE op (VIT_SEGMAX: running max of
  (TRW + alpha_bcast) with a per-cl-page reset via a hand-built
  SUB_DIM_DONE step state) -> page maxes at scr[:, :, 127]; stage =
  m + pot; 8x stream_shuffle -> ALPHA_P broadcast; AHIST DMA per group.

Backward per t (descending): onehot(tag) via iota is_eq; fp32 PE
  transpose; ONE fp32 selector matmul (exact: 0/1 weights) -> W column;
  fused VIT_ROWMAX (cand = alpha + wcol, accum max) -> m*; fused
  VIT_FIRSTIDX (first p with cand==m*, ties lowest) -> tag.
"""
from contextlib import ExitStack

import numpy as np

import concourse.bass as bass
import concourse.tile as tile
from concourse import mybir
from concourse import dve_spec as DS
from concourse import dve_ops as DO
from concourse.dve_spec import (
    Spec, Src0, Src1, C0, C1, Zero, MaxNeg, AluOp, scan, select, eq, Idx,
)
from concourse.dve_uop import DveOpSpec, Trigger

F32 = mybir.dt.float32
ADD = mybir.AluOpType.add
MAX = mybir.AluOpType.max
IS_EQ = mybir.AluOpType.is_equal
NEG_BIG = float(np.float32(-3.0e38))


# ---------------------------------------------------------------- custom ops
def _lower_segmax(spec, ver):
    """lower() with a hand-built FSM: seed -> steady <-> (SUB_DIM_DONE) step,
    where the step state re-seeds the scan accumulator with the current
    element (MAX(MaxNeg, expr)) so the fold restarts at each page."""
    DS._validate_body(spec, ver)
    spec2 = DS._hoist_stream_invariant_ops(spec)
    scans = DS._collect(spec2.body, DS.Scan)
    latches = DS._collect(spec2.body, DS.Latch)
    assert len(scans) == 1 and not latches
    n_lanes, n_stages = DS.N_LANES[ver], DS.N_STAGES[ver]
    p = DS._build_placement(spec2, scans, n_stages, n_lanes)
    seed_ov, _ = DS._scan_overrides(scans, p.node_stage)
    sc = scans[0]
    d = p.node_stage[sc]
    step_ov = {d: DS._Stage(sc.op, MaxNeg, sc.expr)}
    body_lvs = DS._body_scan_leaves(spec2)
    consume = (Src0 in body_lvs, Src1 in body_lvs)
    states = [
        DS._State(placement=p, overrides=seed_ov, trigger=DS.COUNT_ONCE,
                  repeat=1, next=(1, 0, 0), write_out=False),
        DS._State(placement=p, consume=consume,
                  trigger=(Trigger.SRC_TENSOR_DONE, Trigger.SUB_DIM_DONE,
                           Trigger.NONE),
                  next=(0, 2, 0)),
        DS._State(placement=p, consume=consume, overrides=step_ov,
                  trigger=(Trigger.SRC_TENSOR_DONE, Trigger.SUB_DIM_DONE,
                           Trigger.COUNT),
                  next=(0, 2, 1), repeat=1),
    ]
    uops = [DS._assemble(s) for s in states]
    for u in uops:
        u.validate(ver)
    return uops


def _register(op, uops_by_ver=None):
    if any(o.name == op.name for o in DO.OPS):
        return
    DO.OPS.append(op)
    DO.CUSTOM_DVE_SPECS[op.name] = op.spec
    row = DO._CUSTOM_DVE_ROW_BASE + len(DO.OPS) - 1
    assert row < 0x20
    DO._SUB_OPCODE_FOR_NAME[op.name] = row
    if uops_by_ver:
        for ver, uops in uops_by_ver.items():
            DO._COMPILE_CACHE[(op.name, ver)] = DveOpSpec(
                name=op.name, opcode=row, uops=uops,
                rd1_en=DS._has_src1(op.spec))


def _sha_for(spec, ver):
    s = DveOpSpec(name="tmp", opcode=1, uops=DS.lower(spec, ver=ver),
                  rd1_en=DS._has_src1(spec))
    return s.sha(ver)


_OPS_CACHE = {}


def get_ops():
    if _OPS_CACHE:
        return _OPS_CACHE
    ver = "v3"

    segmax_spec = Spec(
        body=scan(AluOp.MAX, Src0 + Src1),
        reference=lambda in0, in1, s0, s1, imm2: np.maximum.accumulate(
            (in0.astype(np.float32) + in1), axis=-1))
    segmax = DO.DveOp("VIT_SEGMAX", segmax_spec, subdim=True, uops_sha={})
    _register(segmax, {ver: _lower_segmax(segmax_spec, ver)})

    def _ref_rowmax(in0, in1, c0, c1, c2):
        b = (in0.astype(np.float32) + in1).astype(np.float32)
        m = np.maximum(c0, b.reshape(b.shape[0], -1).max(axis=-1, keepdims=True))
        return b, m

    rowmax_spec = Spec(body=Src0 + Src1, accum=AluOp.MAX, accum_init=C0,
                       reference=_ref_rowmax)
    rowmax = DO.DveOp("VIT_ROWMAX", rowmax_spec, subdim=False,
                      uops_sha={ver: None})
    rowmax.uops_sha[ver] = _sha_for(rowmax_spec, ver)
    _register(rowmax)

    def _ref_firstidx(in0, in1, c0, c1, c2):
        P = in0.shape[0]
        x = in0.reshape(P, -1)
        idx = np.broadcast_to(np.arange(x.shape[1], dtype=np.float32), x.shape)
        masked = np.where(x == c0, idx, c1)
        return masked, np.minimum(c1, masked.min(axis=-1, keepdims=True))

    firstidx_spec = Spec(body=select(eq(Src0, C0), Idx + Zero, C1),
                         accum=AluOp.MIN, accum_init=C1,
                         reference=_ref_firstidx)
    firstidx = DO.DveOp("VIT_FIRSTIDX", firstidx_spec, subdim=False,
                        uops_sha={ver: None})
    firstidx.uops_sha[ver] = _sha_for(firstidx_spec, ver)
    _register(firstidx)

    _OPS_CACHE.update(segmax=segmax, rowmax=rowmax, firstidx=firstidx)
    return _OPS_CACHE


# ------------------------------------------------------------------ utility
def legalize_waits(nc):
    """This container's walrus accepts at most ONE sync wait per
    instruction; Tile emits drains/noops with many.  Split them into
    single-wait NoOps on the same engine."""
    n_split = 0
    for f in nc.m.functions:
        for blk in f.blocks:
            new = []
            for inst in blk.instructions:
                si = inst.sync_info
                if si is not None and si.on_wait and len(si.on_wait) > 1:
                    waits = list(si.on_wait)
                    for j, w in enumerate(waits[:-1]):
                        new.append(mybir.InstNoOp(
                            name=f"{inst.name}-sw{j}", engine=inst.engine,
                            sync_info=mybir.SyncInfo(on_wait=[w], on_update=[])))
                        n_split += 1
                    inst.sync_info = mybir.SyncInfo(
                        on_wait=[waits[-1]], on_update=list(si.on_update))
                new.append(inst)
            blk.instructions = new
    return n_split


def host_prep(inputs_np, trans_np, n_cores=8):
    """Full inputs -> per-core input maps (list of dicts)."""
    B, T, C = inputs_np.shape
    assert C == 128 and B % n_cores == 0
    bl = B // n_cores  # 16

    transT = np.ascontiguousarray(trans_np.T).astype(np.float32)  # [c, q]

    # TRW[b*8+ch, cl, q] = trans[q, ch*16+cl] = transT[ch*16+cl, q]
    trw = np.tile(transT.reshape(8, 16, 128)[None], (bl, 1, 1, 1))
    trw = np.ascontiguousarray(trw.reshape(128, 16 * 128), dtype=np.float32)

    iota = np.ascontiguousarray(
        np.tile(np.arange(128, dtype=np.float32)[None, :], (16, 1)))
    ident = np.eye(16, dtype=np.float32)

    in_maps = []
    for core in range(n_cores):
        pc = inputs_np[core * bl:(core + 1) * bl]  # [16, T, 128]
        pot = pc.reshape(bl, T, 8, 16).transpose(0, 2, 1, 3)
        pot = np.ascontiguousarray(pot.reshape(128, T * 16), dtype=np.float32)
        in_maps.append({
            "pot": pot, "trw": trw, "wt": transT,
            "iota": iota, "ident": ident,
        })
    return in_maps


def build(T=2048, UF=8, UB=8, legalize=True):
    """Build the Bass program. Returns nc."""
    OPS = get_ops()
    nc = bass.Bass()

    d_pot = nc.dram_tensor("pot", [128, T * 16], F32, kind="ExternalInput")
    d_trw = nc.dram_tensor("trw", [128, 16 * 128], F32, kind="ExternalInput")
    d_wt = nc.dram_tensor("wt", [128, 128], F32, kind="ExternalInput")
    d_iota = nc.dram_tensor("iota", [16, 128], F32, kind="ExternalInput")
    d_ident = nc.dram_tensor("ident", [16, 16], F32, kind="ExternalInput")
    d_tags = nc.dram_tensor("tags", [16, T], F32, kind="ExternalOutput")
    d_ahist = nc.dram_tensor("ahist", [128, T * 16], F32, kind="Internal")
    ahist_bt = d_ahist.rearrange("(b ch) (t cl) -> b t ch cl", ch=8, cl=16)

    with tile.TileContext(nc) as tc, ExitStack() as ctx:
        singles = ctx.enter_context(tc.tile_pool(name="singles", bufs=1))

        # ---------------- forward ----------------
        with (
            tc.tile_pool(name="potp", bufs=1) as potp,
            tc.tile_pool(name="stp", bufs=2) as stp,
            tc.tile_pool(name="scrp", bufs=2) as scrp,
        ):
            s_trw = singles.tile([128, 16, 128], F32)
            s_alpha = singles.tile([128, 128], F32)  # ALPHA_P
            nc.sync.dma_start(
                out=s_trw[:], in_=d_trw.rearrange("p (cl q) -> p cl q", cl=16))

            s_pot = potp.tile([128, T * 16], F32)
            NPC = 8  # split preload so early compute can start sooner
            for c in range(NPC):
                sl = slice(c * T * 16 // NPC, (c + 1) * T * 16 // NPC)
                nc.sync.dma_start(out=s_pot[:, sl], in_=d_pot[:, sl])

            # touch each preload chunk on DVE so the loop body carries no
            # extra DMA-queue waits (back-edge drain has limited wait slots)
            s_touch = singles.tile([128, NPC], F32)
            for c in range(NPC):
                nc.vector.tensor_copy(
                    s_touch[:, c:c + 1],
                    s_pot[:, c * T * 16 // NPC:c * T * 16 // NPC + 1])

            # t=0 init
            st0 = stp.tile([128, UF * 16], F32, tag="stage")
            nc.vector.tensor_copy(st0[:, 0:16], s_pot[:, 0:16])
            for ch in range(8):
                mask = [(j & ~7) | ch for j in range(32)]
                nc.vector.stream_shuffle(
                    s_alpha[:, ch * 16:(ch + 1) * 16], st0[:, 0:16], mask)
            nc.sync.dma_start(out=d_ahist[:, 0:16], in_=st0[:, 0:16])

            def fwd_group(iv0, unroll):
                stage = stp.tile([128, UF * 16], F32, tag="stage")
                for k in range(unroll):
                    iv = iv0 + k * 16
                    scr = scrp.tile([128, 16, 128], F32, tag="scr")
                    alb = s_alpha[:]
                    al_bcast = bass.AP(
                        tensor=alb.tensor, offset=alb.offset,
                        ap=[list(alb.ap[0]), [0, 16], [1, 128]])
                    nc.vector._custom_dve(
                        OPS["segmax"], out=scr[:], in0=s_trw[:], in1=al_bcast)
                    # page maxes at scr[:, :, 127]
                    so = scr[:]
                    m_ap = bass.AP(tensor=so.tensor, offset=so.offset + 127,
                                   ap=[list(so.ap[0]), [128, 16]])
                    ksl = slice(k * 16, (k + 1) * 16)
                    nc.vector.tensor_add(
                        stage[:, ksl], m_ap, s_pot[:, iv:iv + 16])
                    for ch in range(8):
                        mask = [(j & ~7) | ch for j in range(32)]
                        nc.vector.stream_shuffle(
                            s_alpha[:, ch * 16:(ch + 1) * 16], stage[:, ksl],
                            mask)
                nc.sync.dma_start(
                    out=d_ahist[:, iv0:iv0 + unroll * 16],
                    in_=stage[:, 0:unroll * 16])

            ngrp, rem = divmod(T - 1, UF)
            for g in range(ngrp):
                fwd_group(16 + g * UF * 16, UF)
            if rem:
                fwd_group(16 + ngrp * UF * 16, rem)

        # ---------------- backward ----------------
        with (
            tc.tile_pool(name="abp", bufs=2) as abp,
            tc.tile_pool(name="psp", bufs=2, space="PSUM") as psp,
        ):
            s_wt = singles.tile([128, 128], F32)
            nc.sync.dma_start(out=s_wt[:], in_=d_wt[:])
            s_iota = singles.tile([16, 128], F32)
            nc.sync.dma_start(out=s_iota[:], in_=d_iota[:])
            s_ident = singles.tile([16, 16], F32)
            nc.sync.dma_start(out=s_ident[:], in_=d_ident[:])

            s_tags = singles.tile([16, T], F32)
            s_tcol = singles.tile([16, 1], F32)    # current tag per b
            s_mv = singles.tile([16, 1], F32)
            s_oh = singles.tile([16, 128], F32)
            s_oht = singles.tile([128, 16], F32)
            s_cand = singles.tile([16, 128], F32)
            s_junk = singles.tile([16, 128], F32)

            # init: tag_{T-1} = argmax(alpha_{T-1})
            s_alast = singles.tile([16, 1, 8, 16], F32)
            s_zero = singles.tile([16, 128], F32)
            nc.vector.memset(s_zero[:], 0)
            nc.sync.dma_start(out=s_alast[:], in_=ahist_bt[:, T - 1:T, :, :])
            al2 = s_alast[:].rearrange("b t ch cl -> b (t ch cl)")
            nc.vector._custom_dve(
                OPS["rowmax"], out=s_cand[:], in0=al2, in1=s_zero[:],
                s0=NEG_BIG, accum_out=s_mv[:])
            nc.vector._custom_dve(
                OPS["firstidx"], out=s_junk[:], in0=s_cand[:],
                s0=s_mv[:], s1=1.0e9, accum_out=s_tcol[:])
            nc.scalar.copy(s_tags[:, T - 1:T], s_tcol[:])

            def bwd_group(iv0, unroll):
                # iv0 = highest tprev in this group; covers tprev = iv0-k
                stage = abp.tile([16, UB, 8, 16], F32, tag="bstage")
                for ch in range(8):
                    nc.sync.dma_start(
                        out=stage[:, 0:unroll, ch, :],
                        in_=ahist_bt[:, iv0 - (unroll - 1):iv0 + 1, ch, :])
                st2 = stage[:].rearrange("b t ch cl -> b (t ch cl)")
                for k in range(unroll):
                    tprev_col = unroll - 1 - k
                    abf = st2[:, tprev_col * 128:(tprev_col + 1) * 128]
                    # onehot over curs: oh[b, c] = (c == tag_b)
                    nc.vector.tensor_scalar(
                        out=s_oh[:], in0=s_iota[:], scalar1=s_tcol[:],
                        scalar2=None, op0=IS_EQ)
                    ohp = psp.tile([128, 16], F32, tag="ohp")
                    nc.tensor.transpose(ohp[:], s_oh[:], s_ident[:])
                    nc.vector.tensor_copy(s_oht[:], ohp[:])
                    tcp = psp.tile([16, 128], F32, tag="tcp")
                    nc.tensor.matmul(tcp[:], s_oht[:], s_wt[:],
                                     start=True, stop=True)
                    nc.vector._custom_dve(
                        OPS["rowmax"], out=s_cand[:], in0=abf, in1=tcp[:],
                        s0=NEG_BIG, accum_out=s_mv[:])
                    nc.vector._custom_dve(
                        OPS["firstidx"], out=s_junk[:], in0=s_cand[:],
                        s0=s_mv[:], s1=1.0e9, accum_out=s_tcol[:])
                    nc.scalar.copy(s_tags[:, iv0 - k:iv0 - k + 1],
                                   s_tcol[:])

            nbg, brem = divmod(T - 1, UB)
            start = T - 2
            for g in range(nbg):
                bwd_group(start, UB)
                start -= UB
            if brem:
                bwd_group(start, brem)

            nc.sync.dma_start(out=d_tags[:], in_=s_tags[:])

    mybir.codegen_inst_isa_subclasses(nc)
    if legalize:
        legalize_waits(nc)
    return nc


_NC_CACHE = {}


def _get_nc(T):
    if T not in _NC_CACHE:
        _NC_CACHE[T] = build(T=T, UF=8, UB=8)
    return _NC_CACHE[T]


def kernel(inputs, transitions):
    """Full-input Viterbi decode on 8 NeuronCores (data-parallel over batch)."""
    from concourse import bass_utils

    inputs = np.asarray(inputs)
    transitions = np.asarray(transitions)
    B, T, C = inputs.shape
    n_cores = 8
    in_maps = host_prep(inputs, transitions, n_cores=n_cores)
    nc = _get_nc(T)
    res = bass_utils.run_bass_kernel_spmd(
        nc, in_maps, core_ids=list(range(n_cores)))
    tags = np.concatenate([r["tags"] for r in res.results], axis=0)  # [B, T]
    return tags.astype(inputs.dtype)

